# revision 50
# baseline (speedup 1.0000x reference)
"""Single-head causal attention (B=8, T=2048, C=1024, H=64) on 8 TRN2 NeuronCores.

Sharding: data-parallel over batch -- core b computes batch element b. No
collectives. Per core, for x_b [T, C]:
    q = x_b @ Wq / sqrt(H); k = x_b @ Wk; v = x_b @ Wv
    out = softmax(causal(q @ k.T)) @ v

bf16 end-to-end (host casts x/W to bf16; fp32 PSUM accumulation; measured
rel err ~5e-3 vs fp32 reference, tolerance 2e-2).

V_PACK: V projection column-packed (even C-chunks -> array cols 0-63/psum
rows 0-63, odd -> cols 64-127/rows 64-127, concurrent), folded V = Ve + Vo
after a stacked [128,128] PE transpose.
S_PACK: S^T matmul pairs row-tiled (K=64 each, rows 0-63 / 64-127 of the PE
array, concurrent), Q^T duplicated to both partition halves, K^T tiles
alternating halves.
"""

from contextlib import ExitStack

import numpy as np

import concourse.mybir as mybir
import concourse.tile as tile
from concourse import bacc
from concourse.bass_utils import run_bass_kernel_spmd
from concourse.masks import make_identity

B, T, C, H = 8, 2048, 1024, 64
N_CORES = 8
GQ = 512          # q-group width (PSUM bank)
NG = T // GQ      # 4 q-groups
KT = 128          # k-tile size
CC = C // 128     # 8 contraction chunks
NP = T // (2 * KT)  # 8 k-tile pairs
F32 = mybir.dt.float32
BF16 = mybir.dt.bfloat16
EXP = mybir.ActivationFunctionType.Exp

V_PACK = True
S_PACK = True
DEBUG_DUMP = False


def _emit(ctx, tc):
    nc = tc.nc
    xT = nc.dram_tensor("xT", [C, T], BF16, kind="ExternalInput").ap()
    wqkv = nc.dram_tensor("wqkv", [C, 3 * H], BF16, kind="ExternalInput").ap()
    outT = nc.dram_tensor("outT", [H, T], BF16, kind="ExternalOutput").ap()

    const = ctx.enter_context(tc.tile_pool(name="const", bufs=1))
    persist = ctx.enter_context(tc.tile_pool(name="persist", bufs=1))
    vt_pool = ctx.enter_context(tc.tile_pool(name="vt", bufs=2))
    pt_pool = ctx.enter_context(tc.tile_pool(name="pt", bufs=10))
    out_pool = ctx.enter_context(tc.tile_pool(name="outp", bufs=2))
    nrm_pool = ctx.enter_context(tc.tile_pool(name="nrm", bufs=2))
    ps_qk = ctx.enter_context(tc.tile_pool(name="ps_qk", bufs=1, space="PSUM"))
    ps_v = ctx.enter_context(tc.tile_pool(name="ps_v", bufs=1, space="PSUM"))
    ps_s = ctx.enter_context(tc.tile_pool(name="ps_s", bufs=2, space="PSUM"))
    ps_o = ctx.enter_context(tc.tile_pool(name="ps_o", bufs=2, space="PSUM"))

    # x resident in SBUF. Span 0 arrives in chunk-pair pieces so the first
    # projection matmuls start ~9us in; later spans follow (trigger order
    # staggers their transfers behind span 0).
    xt = persist.tile([128, CC, T], BF16)
    xv = xT.rearrange("(c p) t -> p c t", p=128)
    wsb = const.tile([128, CC, 3 * H], BF16)
    wv2 = wqkv.rearrange("(c p) m -> p c m", p=128)
    nc.sync.dma_start(out=wsb[:, 0:4, :], in_=wv2[:, 0:4, :])
    nc.sync.dma_start(out=wsb[:, 4:8, :], in_=wv2[:, 4:8, :])
    for c0 in range(0, CC, 2):
        nc.scalar.dma_start(out=xt[:, c0 : c0 + 2, 0:GQ],
                            in_=xv[:, c0 : c0 + 2, 0:GQ])
    for g in range(1, NG):
        sl = slice(GQ * g, GQ * (g + 1))
        # chain: a 1-elem copy reads the tail of span g-1 and dirties the
        # head of span g, so this DMA starts only after span g-1 lands --
        # serializing the transfers gives each span full DMA bandwidth.
        anchor_c = 3 if g == 1 else CC - 1
        nc.vector.tensor_copy(
            xt[0:1, 0:1, GQ * g : GQ * g + 1],
            xt[0:1, anchor_c : anchor_c + 1, GQ * g - 1 : GQ * g])
        nc.sync.dma_start(out=xt[:, :, sl], in_=xv[:, :, sl])

    if V_PACK:
        # J = [I64; I64]: matmul against J contracts over partitions, both
        # transposing V^T tiles and folding Ve + Vo in one PE instruction.
        jm_f = const.tile([128, H], F32)
        make_identity(nc, jm_f[0:H, :])
        make_identity(nc, jm_f[H:128, :])
        jmat = const.tile([128, H], BF16)
        nc.vector.tensor_copy(jmat[:], jm_f[:])
    else:
        ident_f = const.tile([H, H], F32)
        make_identity(nc, ident_f[:])
        ident = const.tile([H, H], BF16)
        nc.vector.tensor_copy(ident[:], ident_f[:])

    if S_PACK:
        qt2 = persist.tile([128, NG, GQ], BF16)   # Q^T dup rows 0-63 & 64-127
        kt2 = persist.tile([128, NP, KT], BF16)   # even tiles low, odd high
    else:
        qt2 = persist.tile([H, NG, GQ], BF16)
        kt2 = persist.tile([H, T], BF16)
    # natural V tiles + a 64-wide ones BLOCK: the O matmul then emits the
    # softmax denominator replicated across psum partitions 64-127, so
    # normalization needs no partition broadcast.
    vsb = persist.tile([128, T // KT, 2 * H], BF16)
    nc.gpsimd.memset(vsb[:, :, H : 2 * H], 1.0)

    def emit_normalize(g, o_ps):
        rb = nrm_pool.tile([H, GQ], F32)
        rec = nrm_pool.tile([H, GQ], F32)
        osb = out_pool.tile([H, GQ], BF16)
        hw = GQ // 2
        for hh in range(2):
            s2 = slice(hw * hh, hw * (hh + 1))
            nc.scalar.copy(rb[:, s2], o_ps[H:128, s2])
            nc.vector.reciprocal_approx_fast(rec[:, s2], rb[:, s2])
            nc.vector.tensor_mul(osb[:, s2], o_ps[0:H, s2], rec[:, s2])
            nc.sync.dma_start(out=outT[:, GQ * g + hw * hh : GQ * g + hw * (hh + 1)],
                              in_=osb[:, s2])

    pending = None
    for g in range(NG):
        sl = slice(GQ * g, GQ * (g + 1))
        # ---- projections for t-span g ----
        qk_ps = ps_qk.tile([128, GQ], F32)
        v_ps = ps_v.tile([128, GQ], F32)
        for ci in range(CC):
            nc.tensor.matmul(qk_ps[:], wsb[:, ci, 0:128], xt[:, ci, sl],
                             start=(ci == 0), stop=(ci == CC - 1))
        if V_PACK:
            for ci in range(0, CC, 2):
                nc.tensor.matmul(v_ps[0:H, :], wsb[:, ci, 128:192],
                                 xt[:, ci, sl],
                                 start=(ci == 0), stop=(ci == CC - 2),
                                 tile_position=(0, 0), skip_group_check=True)
                nc.tensor.matmul(v_ps[H:128, :], wsb[:, ci + 1, 128:192],
                                 xt[:, ci + 1, sl],
                                 start=(ci == 0), stop=(ci + 1 == CC - 1),
                                 tile_position=(0, 64), skip_group_check=True)
        else:
            for ci in range(CC):
                nc.tensor.matmul(v_ps[0:H, :], wsb[:, ci, 128:192],
                                 xt[:, ci, sl],
                                 start=(ci == 0), stop=(ci == CC - 1))
        # ---- Q/K copies ----
        if S_PACK:
            nc.vector.tensor_copy(qt2[64:128, g, :], qk_ps[0:H, :])
            nc.vector.tensor_copy(qt2[0:64, g, :], qk_ps[0:H, :])
            kev = qk_ps[H:128, :].rearrange("p (m c) -> p m c", c=KT)
            nc.vector.tensor_copy(kt2[0:64, 2 * g : 2 * g + 2, :],
                                  kev[:, 0::2, :])
            nc.vector.tensor_copy(kt2[64:128, 2 * g : 2 * g + 2, :],
                                  kev[:, 1::2, :])
        else:
            nc.scalar.copy(qt2[:, g, :], qk_ps[0:H, :])
            nc.vector.tensor_copy(kt2[:, sl], qk_ps[H:128, :])
        # ---- V -> natural tiles ----
        if V_PACK:
            vt = vt_pool.tile([128, GQ], BF16)
            nc.vector.tensor_copy(vt[:], v_ps[:])
            fo = ps_s.tile([128, 4, H], F32, tag="s")
            for jj in range(4):
                j = 4 * g + jj
                # V tile = vt_chunk.T @ [I64; I64]: transpose + Ve+Vo fold
                nc.tensor.matmul(fo[:, jj, :], vt[:, KT * jj : KT * (jj + 1)],
                                 jmat[:], start=True, stop=True)
                nc.vector.tensor_copy(vsb[:, j, 0:H], fo[:, jj, :])
        else:
            vt = vt_pool.tile([H, GQ], BF16)
            nc.vector.tensor_copy(vt[:], v_ps[0:H, :])
            tr = ps_s.tile([KT, 4, H], BF16, tag="s")
            for jj in range(4):
                j = 4 * g + jj
                nc.tensor.transpose(tr[:, jj, :],
                                    vt[:, KT * jj : KT * (jj + 1)], ident[:])
                nc.scalar.copy(vsb[:, j, 0:H], tr[:, jj, :])
        if g == NG - 1:
            heat = ps_qk.tile([128, GQ], F32, tag="qk_ps")
        # deferred normalize of the previous group: its DVE ops now sit
        # behind this span's critical copies instead of ahead of them.
        if pending is not None:
            emit_normalize(*pending)
            pending = None
        # ---- attention for q-group g ----
        o_ps = ps_o.tile([128, GQ], F32)
        jmax = 4 * g + 3
        for m in range(2 * g + 2):
            j0, j1 = 2 * m, 2 * m + 1
            s0, s1 = j0 - 4 * g, j1 - 4 * g
            qlo0, qlo1 = max(0, KT * s0), max(0, KT * s1)
            sp = ps_s.tile([128, 2 * GQ], F32, tag="s")
            if S_PACK:
                nc.tensor.matmul(sp[:, qlo0:GQ], kt2[0:64, m, :],
                                 qt2[0:64, g, qlo0:GQ],
                                 start=True, stop=True, tile_position=(0, 0))
                nc.tensor.matmul(sp[:, GQ + qlo1 : 2 * GQ], kt2[64:128, m, :],
                                 qt2[64:128, g, qlo1:GQ],
                                 start=True, stop=True, tile_position=(64, 0))
            else:
                nc.tensor.matmul(sp[:, qlo0:GQ], kt2[:, KT * j0 : KT * j0 + KT],
                                 qt2[:, g, qlo0:GQ], start=True, stop=True)
                nc.tensor.matmul(sp[:, GQ + qlo1 : 2 * GQ],
                                 kt2[:, KT * j1 : KT * j1 + KT],
                                 qt2[:, g, qlo1:GQ], start=True, stop=True)
            pt = pt_pool.tile([128, 2 * GQ], BF16)
            if s1 < 0:
                nc.scalar.activation(pt[:], sp[:], EXP)
            else:
                nc.scalar.activation(pt[:, qlo0:GQ], sp[:, qlo0:GQ], EXP)
                nc.scalar.activation(pt[:, GQ + qlo1 : 2 * GQ],
                                     sp[:, GQ + qlo1 : 2 * GQ], EXP)
            # causal mask on the diagonal 128x128 block: keep k_local <= q_local
            for (ss, off) in ((s0, qlo0), (s1, GQ + qlo1)):
                if ss >= 0:
                    nc.gpsimd.affine_select(
                        out=pt[:, off : off + KT], in_=pt[:, off : off + KT],
                        compare_op=mybir.AluOpType.is_ge, fill=0.0,
                        base=0, pattern=[[1, KT]], channel_multiplier=-1)
            if DEBUG_DUMP and ((g, m) in ((3, 0), (0, 0), (0, 1))):
                dbg_pt = nc.dram_tensor(f"dbg_pt_{g}_{m}", [128, 2 * GQ], BF16,
                                        kind="ExternalOutput").ap()
                nc.sync.dma_start(out=dbg_pt, in_=pt[:])
            nc.tensor.matmul(o_ps[:, qlo0:GQ], vsb[:, j0, :], pt[:, qlo0:GQ],
                             start=(j0 == 0), stop=(j0 == jmax))
            nc.tensor.matmul(o_ps[:, qlo1:GQ], vsb[:, j1, :],
                             pt[:, GQ + qlo1 : 2 * GQ],
                             start=(j1 == 0), stop=(j1 == jmax))
            if g == NG - 1 and m < 2 * g:
                nc.tensor.matmul(heat[:], vsb[:, 0, :], pt[:, 0:GQ],
                                 start=True, stop=True, skip_group_check=True)
        if S_PACK and g < NG - 1:
            heatb = ps_s.tile([128, GQ], F32, tag="s")
            for _ in range(3):
                nc.tensor.matmul(heatb[:], vsb[:, 4 * g, :], qt2[:, g, :],
                                 start=True, stop=True, skip_group_check=True)
        pending = (g, o_ps)
    emit_normalize(*pending)

    if DEBUG_DUMP:
        qshape = [128, NG, GQ] if S_PACK else [H, NG, GQ]
        kshape = [128, NP, KT] if S_PACK else [H, T]
        dbg_qt = nc.dram_tensor("dbg_qt", qshape, BF16,
                                kind="ExternalOutput").ap()
        nc.sync.dma_start(out=dbg_qt, in_=qt2[:])
        dbg_kt = nc.dram_tensor("dbg_kt", kshape, BF16,
                                kind="ExternalOutput").ap()
        nc.sync.dma_start(out=dbg_kt, in_=kt2[:])
        dbg_vsb = nc.dram_tensor("dbg_vsb", [128, T // KT, 2 * H], BF16,
                                 kind="ExternalOutput").ap()
        nc.sync.dma_start(out=dbg_vsb, in_=vsb[:])
        dbg_xt = nc.dram_tensor("dbg_xt", [128, CC, T], BF16,
                                kind="ExternalOutput").ap()
        nc.sync.dma_start(out=dbg_xt, in_=xt[:])


def build():
    nc = bacc.Bacc("TRN2", target_bir_lowering=False, debug=False)
    with tile.TileContext(nc) as tc:
        with ExitStack() as ctx:
            _emit(ctx, tc)
    nc.compile()
    return nc


_NC_CACHE = None


def _get_module():
    global _NC_CACHE
    if _NC_CACHE is None:
        _NC_CACHE = build()
    return _NC_CACHE


def prep_in_maps(x, Wq, Wk, Wv):
    import ml_dtypes

    bf = ml_dtypes.bfloat16
    x = np.asarray(x, dtype=np.float32)
    Wq = np.asarray(Wq, dtype=np.float32)
    Wk = np.asarray(Wk, dtype=np.float32)
    Wv = np.asarray(Wv, dtype=np.float32)
    wqkv = np.ascontiguousarray(
        np.concatenate([Wq * (1.0 / np.sqrt(H)), Wk, Wv], axis=1)).astype(bf)
    return [
        {"xT": np.ascontiguousarray(x[b].T).astype(bf), "wqkv": wqkv}
        for b in range(B)
    ]


def assemble_out(results):
    out = np.empty((B, T, H), dtype=np.float32)
    for b in range(B):
        out[b] = np.asarray(results[b]["outT"], dtype=np.float32).T
    return out


def run(x, Wq, Wk, Wv, trace=False):
    nc = _get_module()
    in_maps = prep_in_maps(x, Wq, Wk, Wv)
    res = run_bass_kernel_spmd(nc, in_maps, core_ids=list(range(N_CORES)),
                               trace=trace)
    return assemble_out(res.results), res


def kernel(x, Wq, Wk, Wv):
    out, _ = run(x, Wq, Wk, Wv)
    return out
